# revision 18
# baseline (speedup 1.0000x reference)
"""Causal self-attention (B=2, S=4096, D=512, H=8) on 8 Trainium2 cores.

Sharding: core c handles batch b = c//4 and heads {2*(c%4), 2*(c%4)+1}.

Design (v4): k-major flash attention with an fp8-DoubleRow PV stage.

  - Inner loop is PAIR-BATCHED by PE tiling mode: a [64-row mode: QK x4]
    phase then a [128-row mode: PV + background] phase per TWO key blocks,
    so the tensor-engine drain that accompanies a tiling-mode change is
    paid once per pair instead of twice per block.  The two heads' QK
    matmuls run CONCURRENTLY in the two 64-row tiles.
  - PV for chunks J>=1 runs in fp8e4 DoubleRow mode: one matmul per
    (pair of key blocks, head) at 2 fp8 MACs/cell/cycle.  exp() writes
    e4m3 directly: ScalarE ACTIVATE casts to fp8, VectorE uses an int8
    Schraudolph (uint8(round(A8*score + bias)) IS the e4m3 bit pattern of
    2^(log2e*score/8 + delta); negatives saturate to 0 = +0.0).  V is
    quantized to e4m3 at transpose time.  Chunk 0 (queries 0..511, whose
    rows average over few keys) keeps the full-precision bf16 path - the
    fp8 error is a pure early-row phenomenon.
  - All scores carry a global offset -C (cancels in softmax) keeping
    exp() inside e4m3 range; the per-key dither delta is quantized so
    that 2^-delta is EXACTLY representable in e4m3, making the dither
    cancellation in PV exact.
  - The causal triangle mask is folded into the exp instruction itself
    via scalar_tensor_tensor with a host-built [128,128] bias tile
    (schraudolph bias + 0/-1e9), so no separate mask multiply exists.
    The odd member of a diagonal DoubleRow pair gets its dead 128-column
    strip zeroed by a GPSIMD memset (the only engine with free cycles).
  - Chunks ascend (0..7) as in the baseline: the triangular load ramp
    delays DVFS throttling, which a front-loaded schedule triggers early.

PSUM budget (8 banks): st x4 (single-head score tiles) | pv0 pv1 | bg x2.
"""

import sys

sys.path.insert(0, "/opt/trn_rl_repo")

from contextlib import ExitStack

import ml_dtypes
import numpy as np

import concourse.bass as bass
import concourse.tile as tile
from concourse import bacc, bass_utils, mybir

B, S, D = 2, 4096, 512
H, HD = 8, 64
NCORES = 8
F32 = mybir.dt.float32
BF16 = mybir.dt.bfloat16
I16 = mybir.dt.int16
U8 = mybir.dt.uint8
FP8 = mybir.dt.float8e4
DR = mybir.MatmulPerfMode.DoubleRow
EXP = mybir.ActivationFunctionType.Exp
IDENT = mybir.ActivationFunctionType.Identity
MULT = mybir.AluOpType.mult
ADD = mybir.AluOpType.add
NPBF16 = ml_dtypes.bfloat16
NPFP8 = ml_dtypes.float8_e4m3

CK = 512                      # query-chunk width
NCH = S // CK                 # 8
KBLK = 128                    # key block (partition dim)
KB_PER_CK = CK // KBLK        # 4
NEG = -1.0e30
LOG2E = 1.4426950408889634
A16 = 128 * LOG2E * 0.125     # DVE trick multiplier, bf16 (raw-score units)
A8 = LOG2E                    # DVE trick multiplier, e4m3
CSH = -0.045                  # Schraudolph shift
PHI = 0.6180339887498949
COFF = 24.0                   # global score offset (cancels in softmax)

# fpack column offsets (f32 constants)
BQKV, KB16, BACT16, VSC = 0, 3, 35, 67
TRI16, KB8, BACT8, TRI8 = 68, 196, 228, 260
FPW = 388


class Balancer:
    """Greedy ns-accounting across ACT and DVE for balanceable ops."""

    def __init__(self, nc):
        self.nc = nc
        self.ns = {"act": 2700.0, "dve": 0.0}  # ACT pays the exp table load

    def _cost(self, eng, w):
        return (w + 352) / 1.2 if eng == "act" else (w + 150) / 0.96

    def charge(self, eng, w):
        self.ns[eng] += self._cost(eng, w)

    def pick(self, w):
        eng = "act" if self.ns["act"] + self._cost("act", w) <= \
            self.ns["dve"] + self._cost("dve", w) else "dve"
        self.charge(eng, w)
        return eng


def _emit(nc, tc, ctx, io):
    xT, wpack, fpack, poT = io

    bal = Balancer(nc)

    const = ctx.enter_context(tc.tile_pool(name="const", bufs=1))
    sb = ctx.enter_context(tc.tile_pool(name="sb", bufs=1))

    cb = const.tile([128, 2240], BF16, tag="cbf16")
    cf = const.tile([128, FPW], F32, tag="cf32")
    nc.sync.dma_start(cb[:, 0:512], wpack[:, 0:512])
    nc.sync.dma_start(cb[:, 512:2240], wpack[:, 512:2240])
    nc.sync.dma_start(cf[:, 0:68], fpack[:, 0:68])
    nc.sync.dma_start(cf[:, 68:FPW], fpack[:, 68:FPW])
    W_Q, W_K, W_V, W_O, ID2 = 0, 512, 1024, 1536, 2176

    # ---- persistent SBUF ----
    kT = sb.tile([128, S], BF16, tag="kT")        # [2*64 hd, keys]
    # k-major V: fp8 for the DoubleRow path (all keys), bf16 for chunk 0's
    # keys 0..511.  128-col blocks: [hd(64) | ones@64 | pad], ones = 2^-delta
    v0 = sb.tile([128, 32 * 128], FP8, tag="v0")
    v1 = sb.tile([128, 32 * 128], FP8, tag="v1")
    v0b = sb.tile([128, 4 * 128], BF16, tag="v0b")
    v1b = sb.tile([128, 4 * 128], BF16, tag="v1b")

    xin = ctx.enter_context(tc.tile_pool(name="xin", bufs=2))
    qp = ctx.enter_context(tc.tile_pool(name="qp", bufs=2))
    vtp = ctx.enter_context(tc.tile_pool(name="vtp", bufs=2))
    etp = ctx.enter_context(tc.tile_pool(name="etp", bufs=6))    # bf16, J=0
    e8p = ctx.enter_context(tc.tile_pool(name="e8p", bufs=3))    # fp8 pairs
    otp = ctx.enter_context(tc.tile_pool(name="otp", bufs=2))
    pop = ctx.enter_context(tc.tile_pool(name="pop", bufs=2))
    rdp = ctx.enter_context(tc.tile_pool(name="rdp", bufs=2))

    ps_st = ctx.enter_context(tc.tile_pool(name="ps_st", bufs=1, space="PSUM"))
    ps_pv = ctx.enter_context(tc.tile_pool(name="ps_pv", bufs=1, space="PSUM"))
    ps_bg = ctx.enter_context(tc.tile_pool(name="ps_bg", bufs=2, space="PSUM"))

    # ones columns (scaled 2^-delta, e4m3-exact)
    for vdst in (v0, v1):
        oc = vdst[:].rearrange("p (k c) -> p k c", c=128)[:, :, 64:65]
        nc.vector.tensor_copy(oc, cf[:, VSC:VSC + 1].to_broadcast((128, 32, 1)))
    for vdst in (v0b, v1b):
        oc = vdst[:].rearrange("p (k c) -> p k c", c=128)[:, :, 64:65]
        nc.vector.tensor_copy(oc, cf[:, VSC:VSC + 1].to_broadcast((128, 4, 1)))

    # ---------------- background tasks, split by PE tiling mode ----------
    bg64, bg128 = [], []
    pace = {"c64": 0.0, "r64": 0.0, "c128": 0.0, "r128": 0.0}

    def drain64():
        pace["c64"] += pace["r64"]
        while pace["c64"] >= 1.0 and bg64:
            pace["c64"] -= 1.0
            bg64.pop(0)()

    def drain128():
        pace["c128"] += pace["r128"]
        while pace["c128"] >= 1.0 and bg128:
            pace["c128"] -= 1.0
            bg128.pop(0)()

    def copy_psum(dst_ap, src_ap, w, bias_col=None, scale=1.0):
        eng = bal.pick(w)
        if eng == "act":
            if bias_col is not None:
                nc.scalar.activation(dst_ap, src_ap, IDENT, bias=bias_col,
                                     scale=scale)
            else:
                nc.scalar.copy(dst_ap, src_ap)
        else:
            if bias_col is not None:
                nc.vector.tensor_scalar(dst_ap, src_ap, scale, bias_col,
                                        MULT, ADD)
            else:
                nc.vector.tensor_copy(dst_ap, src_ap)

    q_tiles = {}

    def emit_proj(J):
        """q/k/v projections for chunk J (bg128) + V transposes (bg64)."""
        xt = xin.tile([128, 4 * CK], BF16, tag="x")
        for ks in range(4):
            nc.sync.dma_start(
                xt[:, ks * CK:(ks + 1) * CK],
                xT[ks * 128:(ks + 1) * 128, J * CK:(J + 1) * CK])
        qt = qp.tile([128, CK], BF16, tag="q")
        q_tiles[J] = qt
        vt = vtp.tile([128, CK], BF16, tag="v")
        csl = slice(J * CK, (J + 1) * CK)

        def mk_proj(woff, bcol, dst_ap):
            def f():
                ps = ps_bg.tile([128, CK], F32, tag="bg")
                for ks in range(4):
                    nc.tensor.matmul(
                        ps[:], cb[:, woff + ks * 128:woff + (ks + 1) * 128],
                        xt[:, ks * CK:(ks + 1) * CK],
                        start=(ks == 0), stop=(ks == 3))
                copy_psum(dst_ap, ps[:], CK,
                          bias_col=cf[:, BQKV + bcol:BQKV + bcol + 1])
            return f

        bg128.append(mk_proj(W_Q, 0, qt[:]))
        bg128.append(mk_proj(W_K, 1, kT[:, csl]))
        bg128.append(mk_proj(W_V, 2, vt[:]))

        def mk_vtrans(hh, vdst, vdstb):
            def f():
                ps = ps_bg.tile([128, CK], F32, tag="bg")
                tr = ps[:].bitcast(BF16)
                for i in range(4):
                    nc.tensor.transpose(
                        tr[:, i * 64:(i + 1) * 64],
                        vt[hh * 64:(hh + 1) * 64, i * KBLK:(i + 1) * KBLK],
                        cb[hh * 64:(hh + 1) * 64, ID2:ID2 + 64])
                dst = vdst[:, (J * 4) * 128:(J * 4 + 4) * 128]
                dst = dst.rearrange("p (k c) -> p k c", c=128)[:, :, 0:64]
                src = tr[:, 0:256].rearrange("p (k c) -> p k c", c=64)
                nc.vector.tensor_scalar_mul(dst, src, cf[:, VSC:VSC + 1])
                bal.charge("dve", 256)
                if vdstb is not None:  # chunk 0 also needs bf16 V
                    dstb = vdstb[:].rearrange("p (k c) -> p k c", c=128)[:, :, 0:64]
                    nc.vector.tensor_scalar_mul(dstb, src, cf[:, VSC:VSC + 1])
                    bal.charge("dve", 256)
            return f
        # transposes must not interleave with 64-row-tiled QK matmuls (the
        # mode interaction corrupts them) - keep them in the 128-mode phase
        bg128.append(mk_vtrans(0, v0, v0b if J == 0 else None))
        bg128.append(mk_vtrans(1, v1, v1b if J == 0 else None))

    oT_tiles = {}

    def emit_div(J, pv0t, pv1t):
        oT = otp.tile([128, CK], BF16, tag="oT")
        oT_tiles[J] = oT
        rdB = []
        for hh, pvt in ((0, pv0t), (1, pv1t)):
            den_t = rdp.tile([1, CK], F32, tag=f"den{hh}")
            nc.vector.tensor_copy(den_t[:], pvt[64:65, :])
            rd = rdp.tile([1, CK], F32, tag=f"rd{hh}")
            nc.vector.reciprocal_approx_fast(rd[:], den_t[:])
            bal.charge("dve", 2 * CK)
            rb = rdp.tile([64, CK], F32, tag=f"rdB{hh}")
            nc.gpsimd.partition_broadcast(rb[:], rd[:], channels=64)
            rdB.append(rb)
        for hh, pvt in ((0, pv0t), (1, pv1t)):
            hsl = slice(hh * 64, (hh + 1) * 64)
            nc.vector.tensor_mul(oT[hsl, :], pvt[0:64, :], rdB[hh][:])
            bal.charge("dve", CK)

    def emit_outproj(J):
        oT = oT_tiles.pop(J)

        def mk(dt_):
            def f():
                ps = ps_bg.tile([128, CK], F32, tag="bg")
                nc.tensor.matmul(
                    ps[:], cb[:, W_O + dt_ * 128:W_O + (dt_ + 1) * 128],
                    oT[:], start=True, stop=True)
                po = pop.tile([128, CK], BF16, tag="po")
                copy_psum(po[:], ps[:], CK)
                nc.sync.dma_start(
                    poT[dt_ * 128:(dt_ + 1) * 128, J * CK:(J + 1) * CK],
                    po[:])
            return f
        for dt_ in range(4):
            bg128.append(mk(dt_))

    # ---------------- exp emitters ----------------
    def emit_exp16(kb, st, et, col0, diag):
        """bf16 exp (chunk 0): schraudolph int16 / ACT exp."""
        if diag:
            nc.vector.scalar_tensor_tensor(
                et[:, col0:col0 + KBLK].bitcast(I16), st[:, col0:col0 + KBLK],
                float(A16), cf[:, TRI16:TRI16 + KBLK], MULT, ADD)
            bal.charge("dve", KBLK)
            col0 += KBLK
            if col0 >= CK:
                return
        if bal.pick(CK - col0) == "act":
            nc.scalar.activation(
                et[:, col0:], st[:, col0:], EXP,
                bias=cf[:, BACT16 + kb:BACT16 + kb + 1], scale=0.125)
        else:
            nc.vector.tensor_scalar(
                et[:, col0:].bitcast(I16), st[:, col0:],
                float(A16), cf[:, KB16 + kb:KB16 + kb + 1], MULT, ADD)

    def emit_exp8(kb, st, et8, half, col0, diag):
        """e4m3 exp into half of a DoubleRow pair tile."""
        base = half * CK
        if diag:
            nc.vector.scalar_tensor_tensor(
                et8[:, base + col0:base + col0 + KBLK].bitcast(U8),
                st[:, col0:col0 + KBLK],
                float(A8), cf[:, TRI8:TRI8 + KBLK], MULT, ADD)
            bal.charge("dve", KBLK)
            col0 += KBLK
            if col0 >= CK:
                return
        if bal.pick(CK - col0) == "act":
            nc.scalar.activation(
                et8[:, base + col0:base + CK], st[:, col0:], EXP,
                bias=cf[:, BACT8 + kb:BACT8 + kb + 1], scale=0.125)
        else:
            nc.vector.tensor_scalar(
                et8[:, base + col0:base + CK].bitcast(U8), st[:, col0:],
                float(A8), cf[:, KB8 + kb:KB8 + kb + 1], MULT, ADD)

    # ---------------- main pipeline ----------------
    emit_proj(0)
    for _ in range(5):
        bg128.pop(0)()

    for J in range(NCH):
        if J + 1 < NCH:
            emit_proj(J + 1)
        if J >= 1:
            emit_outproj(J - 1)
        nkb = KB_PER_CK * (J + 1)
        npair = nkb // 2
        qt = q_tiles.pop(J)
        pv0t = ps_pv.tile([128, CK], F32, tag="pv0")
        pv1t = ps_pv.tile([128, CK], F32, tag="pv1")
        pace["r64"] = (len(bg64) + 0.5) / max(npair, 1)
        pace["r128"] = (len(bg128) + 0.5) / max(npair, 1)
        pair_ets = {}

        def emit_pv_pair(p, last):
            if J == 0:  # bf16 path, per-kb matmuls
                ets = pair_ets.pop(p)
                for i, kb in enumerate((2 * p, 2 * p + 1)):
                    pp = kb - KB_PER_CK * J
                    c0 = KBLK * pp if pp >= 0 else 0
                    for hh, vsb, pv in ((0, v0b, pv0t), (1, v1b, pv1t)):
                        nc.tensor.matmul(
                            pv[:, c0:], vsb[:, kb * 128:(kb + 1) * 128],
                            ets[i][hh][:, c0:],
                            start=(kb == 0), stop=(kb == nkb - 1))
            else:       # fp8 DoubleRow path, per-pair matmuls
                e8s = pair_ets.pop(p)
                pA = 2 * p - KB_PER_CK * J
                c0 = KBLK * pA if pA >= 0 else 0
                for hh, vsb, pv in ((0, v0, pv0t), (1, v1, pv1t)):
                    lhsT = vsb[:, 2 * p * 128:(2 * p + 2) * 128].rearrange(
                        "k (two c) -> k two c", two=2)[:, :, 0:65]
                    rhs = e8s[hh][:].rearrange(
                        "p (two n) -> p two n", two=2)[:, :, c0:]
                    nc.tensor.matmul(
                        pv[0:65, c0:], lhsT, rhs,
                        start=(p == 0), stop=last, perf_mode=DR)

        for p in range(npair):
            # ---- 64-row mode phase: QK x4 ----
            exps = []
            sts = {}
            for kb in (2 * p, 2 * p + 1):
                pp = kb - KB_PER_CK * J
                c0 = KBLK * pp if pp >= 0 else 0
                for hh in range(2):
                    st = ps_st.tile([128, CK], F32, tag=f"st{hh}{kb % 2}")
                    hsl = slice(hh * 64, (hh + 1) * 64)
                    nc.tensor.matmul(
                        st[:, c0:], kT[hsl, kb * KBLK:(kb + 1) * KBLK],
                        qt[hsl, c0:], start=True, stop=True)
                    sts[(kb, hh)] = (st, c0, pp >= 0)
            drain64()
            # ---- exp phase ----
            if J == 0:
                ets = []
                for kb in (2 * p, 2 * p + 1):
                    he = []
                    for hh in range(2):
                        st, c0, diag = sts[(kb, hh)]
                        et = etp.tile([128, CK], BF16, tag="et")
                        emit_exp16(kb, st, et, c0, diag)
                        he.append(et)
                    ets.append(he)
                pair_ets[p] = ets
            else:
                e8a = e8p.tile([128, 2 * CK], FP8, tag="e80")
                e8b = e8p.tile([128, 2 * CK], FP8, tag="e81")
                e8s = [e8a, e8b]
                pA = 2 * p - KB_PER_CK * J
                c0p = KBLK * pA if pA >= 0 else 0
                for half, kb in enumerate((2 * p, 2 * p + 1)):
                    for hh in range(2):
                        st, c0, diag = sts[(kb, hh)]
                        emit_exp8(kb, st, e8s[hh], half, c0, diag)
                        if half == 1 and diag:
                            # dead strip of the odd DoubleRow member
                            nc.gpsimd.memset(
                                e8s[hh][:, CK + c0p:CK + c0p + KBLK], 0.0)
                pair_ets[p] = e8s
            # ---- 128-row mode phase: PV of previous pair + bg ----
            if p >= 1:
                emit_pv_pair(p - 1, last=False)
            drain128()
        emit_pv_pair(npair - 1, last=True)
        emit_div(J, pv0t, pv1t)

    emit_outproj(NCH - 1)
    while bg64:
        bg64.pop(0)()
    while bg128:
        bg128.pop(0)()


_CACHED = None


def _build():
    global _CACHED
    if _CACHED is not None:
        return _CACHED
    nc = bacc.Bacc("TRN2", target_bir_lowering=False, debug=False,
                   enable_asserts=False, num_devices=NCORES)
    names = [
        ("xT", [D, S], BF16), ("wpack", [128, 2240], BF16),
        ("fpack", [128, FPW], F32),
    ]
    aps = [nc.dram_tensor(n, sh, dt_, kind="ExternalInput").ap()
           for n, sh, dt_ in names]
    poT = nc.dram_tensor("poT", [D, S], BF16, kind="ExternalOutput").ap()
    with tile.TileContext(nc) as tc, ExitStack() as ctx:
        _emit(nc, tc, ctx, aps + [poT])
    nc.compile()
    _CACHED = nc
    return nc


def _host_inputs(x, attention_mask, Wq, bq, Wk, bk, Wv, bv, Wo, bo):
    f = np.float32
    x = np.asarray(x, f)
    mask = np.asarray(attention_mask)
    Wq, Wk, Wv, Wo = (np.asarray(w, f) for w in (Wq, Wk, Wv, Wo))
    bq, bk, bv = (np.asarray(b_, f) for b_ in (bq, bk, bv))
    id2 = np.tile(np.eye(64, dtype=NPBF16), (2, 1))
    # dither, quantized so 2^-delta is e4m3-exact (cancellation is exact)
    draw = ((np.arange(128) * PHI) % 1.0).astype(f)
    vs_q = (2.0 ** -draw).astype(NPFP8).astype(f)     # e4m3 grid in [0.5, 1]
    delta = -np.log2(vs_q).astype(f)                  # effective dither
    vscale = vs_q[:, None]
    tri_keep = np.triu(np.ones((128, 128), bool))     # [k, q]: q >= k
    ln2 = np.log(2.0)
    in_maps = []
    for c in range(NCORES):
        b = c // 4
        h0 = 2 * (c % 4)
        hsl = slice(64 * h0, 64 * h0 + 128)

        def pack_w(W):
            wt = W[hsl, :].T
            return np.ascontiguousarray(
                wt.reshape(4, 128, 128).transpose(1, 0, 2)
                .reshape(128, 512).astype(NPBF16))

        wo_t = Wo[:, hsl].T.astype(NPBF16)
        mk = np.where(mask[b] != 0, f(0.0), f(NEG)).astype(f)
        mk = mk.reshape(32, 128).T                    # [128 part, 32 kb]
        mneg = np.where(mk < 0, f(-1e9), f(0.0))
        kb16 = (128.0 * (127.0 + CSH) + 128.0 * delta - A16 * COFF)[:, None] + mneg
        bact = (delta * ln2 - 0.125 * COFF)[:, None] + mk
        kb8 = (8.0 * (7.0 + CSH) + 8.0 * delta - A8 * COFF)[:, None] + mneg
        tri16 = np.where(tri_keep, kb16[:, 0:1], f(-1e9)).astype(f)
        tri8 = np.where(tri_keep, kb8[:, 0:1], f(-1e9)).astype(f)
        wpack = np.concatenate(
            [pack_w(Wq), pack_w(Wk), pack_w(Wv), wo_t,
             np.zeros((128, 128), NPBF16), id2], axis=1)
        fpack = np.concatenate(
            [np.stack([bq[hsl], bk[hsl], bv[hsl]], axis=1).astype(f),
             kb16.astype(f), bact.astype(f), vscale,
             tri16, kb8.astype(f), bact.astype(f), tri8], axis=1)
        assert fpack.shape[1] == FPW

        in_maps.append({
            "xT": np.ascontiguousarray(x[b].T.astype(NPBF16)),
            "wpack": np.ascontiguousarray(wpack),
            "fpack": np.ascontiguousarray(fpack),
        })
    return in_maps


def _assemble(results, bo):
    out = np.zeros((B, S, D), np.float32)
    for c in range(NCORES):
        out[c // 4] += results[c]["poT"].astype(np.float32).T
    out += np.asarray(bo, np.float32)
    return out


def kernel(**inputs) -> np.ndarray:
    nc = _build()
    in_maps = _host_inputs(**inputs)
    last_err = None
    for attempt in range(3):
        try:
            res = bass_utils.run_bass_kernel_spmd(
                nc, in_maps, core_ids=list(range(NCORES)))
            out = _assemble(res.results, inputs["bo"])
        except Exception as e:  # transient NRT/axon device errors
            last_err = e
            continue
        if np.isfinite(out).all():
            return out
        last_err = RuntimeError("non-finite output")
    raise last_err


def run_traced(inputs, **kwargs):
    """test.py helper: run with NTFF tracing, return (out, BassKernelResults)."""
    nc = _build()
    in_maps = _host_inputs(**inputs)
    res = bass_utils.run_bass_kernel_spmd(
        nc, in_maps, core_ids=list(range(NCORES)), trace=True, **kwargs)
    return _assemble(res.results, inputs["bo"]), res


# revision 21
# speedup vs baseline: 1.2952x; 1.2952x over previous
"""Causal self-attention (B=2, S=4096, D=512, H=8) on 8 Trainium2 cores.

Sharding: core c handles batch b = c//4 and heads {2*(c%4), 2*(c%4)+1}.

Design (v2): k-major flash-style attention with the exp() wall split across
TWO engines:
  - ScalarE computes exp natively (ACTIVATE, ~(N+352)/1.2 ns).
  - VectorE computes a one-instruction Schraudolph exp: writing
    int16(round(A*score + bias)) whose bit pattern IS the bf16 of
    2^(log2e*score/8 + delta): the exponent-bit trick computed directly in
    the >>16 scale.  Per-key exponent dither delta_r decorrelates the
    interpolation error; V rows (and the den ones-column) are pre-scaled by
    2^-delta_r on the host so the dither cancels exactly in PV.
A greedy ns-balancer assigns each score tile's exp (and the psum->sbuf
copies) to whichever of ACT/DVE is less loaded, so both engines run ~full
tilt alongside the TensorE stream.

Attention runs in 512-wide query chunks; projections for chunk J+1, the
output projection for chunk J-1, V transposes, and DMA are emitted as
background tasks interleaved between attention steps so PE never idles
(keeps the HAM clock at 2.4 GHz).  Denominators ride the PV matmul as a
65th 'ones' row; oT is divided on-device (reciprocal_approx_fast + gpsimd
partition broadcast) so the two heads fold into ONE output-projection pass
and the core writes a single [512, S] bf16 partial that the host sums.

PSUM budget (8 banks): pv0 pv1 | st x4 (score tiles, f32) | bg x2 (shared
by projections / V-transpose / out-projection).
"""

import sys

sys.path.insert(0, "/opt/trn_rl_repo")

from contextlib import ExitStack

import ml_dtypes
import numpy as np

import concourse.bass as bass
import concourse.tile as tile
from concourse import bacc, bass_utils, mybir

B, S, D = 2, 4096, 512
H, HD = 8, 64
NCORES = 8
F32 = mybir.dt.float32
BF16 = mybir.dt.bfloat16
I16 = mybir.dt.int16
FP8 = mybir.dt.float8e4
DR = mybir.MatmulPerfMode.DoubleRow
NPFP8 = ml_dtypes.float8_e4m3
EXP = mybir.ActivationFunctionType.Exp
IDENT = mybir.ActivationFunctionType.Identity
COPYF = mybir.ActivationFunctionType.Copy
MULT = mybir.AluOpType.mult
ADD = mybir.AluOpType.add
NPBF16 = ml_dtypes.bfloat16

CK = 512                      # query-chunk width
NCH = S // CK                 # 8
KBLK = 128                    # key block (partition dim)
KB_PER_CK = CK // KBLK        # 4
NEG = -1.0e30
LOG2E = 1.4426950408889634
A128 = 128 * LOG2E * 0.125    # DVE trick multiplier (raw-score units)
CSH = -0.045                  # Schraudolph shift
PHI = 0.6180339887498949


class Balancer:
    """Greedy ns-accounting across ACT and DVE for balanceable ops."""

    def __init__(self, nc):
        self.nc = nc
        self.ns = {"act": 2700.0, "dve": 0.0}  # ACT pays the exp table load

    def _cost(self, eng, w):
        return (w + 352) / 1.2 if eng == "act" else (w + 150) / 0.96

    def charge(self, eng, w):
        self.ns[eng] += self._cost(eng, w)

    def pick(self, w):
        eng = "act" if self.ns["act"] + self._cost("act", w) <= \
            self.ns["dve"] + self._cost("dve", w) else "dve"
        self.charge(eng, w)
        return eng


def _emit(nc, tc, ctx, io):
    xT, wpack, fpack, poT = io

    bal = Balancer(nc)

    const = ctx.enter_context(tc.tile_pool(name="const", bufs=1))
    sb = ctx.enter_context(tc.tile_pool(name="sb", bufs=1))

    # ---- constants / weights (two packed DMAs to keep the lead-in short) ----
    cb = const.tile([128, 2240], BF16, tag="cbf16")
    cf = const.tile([128, 196], F32, tag="cf32")
    nc.sync.dma_start(cb[:, 0:512], wpack[:, 0:512])
    nc.sync.dma_start(cb[:, 512:2240], wpack[:, 512:2240])
    nc.sync.dma_start(cf[:], fpack[:])
    W_Q, W_K, W_V, W_O, TRI, ID2 = 0, 512, 1024, 1536, 2048, 2176
    BQKV, KB23, BACT, VSC, TRI16 = 0, 3, 35, 67, 68

    # ---- persistent SBUF ----
    kT = sb.tile([128, S], BF16, tag="kT")       # [2*64 hd, keys]
    # k-major V blocks padded to 128 cols (hd | ones@64 | junk pad) so the
    # PV LDWEIGHTS takes the full-width fast path; pv rows 65+ are junk
    v0 = sb.tile([128, 32 * 128], BF16, tag="v0")
    v1 = sb.tile([128, 32 * 128], BF16, tag="v1")

    xin = ctx.enter_context(tc.tile_pool(name="xin", bufs=2))
    qp = ctx.enter_context(tc.tile_pool(name="qp", bufs=2))
    vtp = ctx.enter_context(tc.tile_pool(name="vtp", bufs=2))
    etp = ctx.enter_context(tc.tile_pool(name="etp", bufs=6))
    otp = ctx.enter_context(tc.tile_pool(name="otp", bufs=2))
    pop = ctx.enter_context(tc.tile_pool(name="pop", bufs=4))
    rdp = ctx.enter_context(tc.tile_pool(name="rdp", bufs=2))

    ps_pv = ctx.enter_context(tc.tile_pool(name="ps_pv", bufs=1, space="PSUM"))
    ps_st = ctx.enter_context(tc.tile_pool(name="ps_st", bufs=1, space="PSUM"))
    ps_bg = ctx.enter_context(tc.tile_pool(name="ps_bg", bufs=2, space="PSUM"))

    # ones columns of v0/v1 (scaled 2^-delta); written once, blocks fill later
    for vdst in (v0, v1):
        ones_col = vdst[:].rearrange("p (k c) -> p k c", c=128)[:, :, 64:65]
        nc.vector.tensor_copy(ones_col, cf[:, VSC:VSC + 1].to_broadcast((128, 32, 1)))

    # ---------------- background task machinery ----------------
    bg_tasks = []
    pace = {"credit": 0.0, "rate": 1.0}

    def drain(n):
        for _ in range(min(n, len(bg_tasks))):
            bg_tasks.pop(0)()

    def drain_paced():
        """Spread queued tasks over the chunk's drain slots so the PE always
        has background work, even late in a chunk."""
        pace["credit"] += pace["rate"]
        while pace["credit"] >= 1.0 and bg_tasks:
            pace["credit"] -= 1.0
            bg_tasks.pop(0)()

    def copy_psum(dst_ap, src_ap, w, bias_col=None, scale=1.0):
        """psum->sbuf evacuation on the less-loaded of ACT/DVE."""
        eng = bal.pick(w)
        if eng == "act":
            if bias_col is not None:
                nc.scalar.activation(dst_ap, src_ap, IDENT, bias=bias_col,
                                     scale=scale)
            else:
                nc.scalar.copy(dst_ap, src_ap)
        else:
            if bias_col is not None:
                nc.vector.tensor_scalar(dst_ap, src_ap, scale, bias_col,
                                        MULT, ADD)
            else:
                nc.vector.tensor_copy(dst_ap, src_ap)

    q_tiles = {}

    def emit_proj(J):
        """q/k/v projections for chunk J + V transpose to k-major."""
        xt = xin.tile([128, 4 * CK], BF16, tag="x")
        for ks in range(4):
            nc.sync.dma_start(
                xt[:, ks * CK:(ks + 1) * CK],
                xT[ks * 128:(ks + 1) * 128, J * CK:(J + 1) * CK])
        qt = qp.tile([128, CK], BF16, tag="q")
        q_tiles[J] = qt
        vt = vtp.tile([128, CK], BF16, tag="v")
        csl = slice(J * CK, (J + 1) * CK)

        def mk_proj(woff, bcol, dst_ap):
            def f():
                ps = ps_bg.tile([128, CK], F32, tag="bg")
                for ks in range(4):
                    nc.tensor.matmul(
                        ps[:], cb[:, woff + ks * 128:woff + (ks + 1) * 128],
                        xt[:, ks * CK:(ks + 1) * CK],
                        start=(ks == 0), stop=(ks == 3))
                copy_psum(dst_ap, ps[:], CK,
                          bias_col=cf[:, BQKV + bcol:BQKV + bcol + 1])
            return f

        bg_tasks.append(mk_proj(W_Q, 0, qt[:]))
        bg_tasks.append(mk_proj(W_K, 1, kT[:, csl]))
        bg_tasks.append(mk_proj(W_V, 2, vt[:]))

        def mk_vtrans(hh, vdst):
            def f():
                # own psum buffer per head: a shared bank would let head0's
                # DVE copy (bank read) overlap head1's PE transposes (bank
                # write) -> fatal PSUM collision
                ps = ps_bg.tile([128, CK], F32, tag="bg")
                tr = ps[:].bitcast(BF16)  # [128, 1024] bf16 view
                for i in range(4):
                    nc.tensor.transpose(
                        tr[:, i * 64:(i + 1) * 64],
                        vt[hh * 64:(hh + 1) * 64, i * KBLK:(i + 1) * KBLK],
                        cb[hh * 64:(hh + 1) * 64, ID2:ID2 + 64])
                dst = vdst[:, (J * 4) * 128:(J * 4 + 4) * 128]
                dst = dst.rearrange("p (k c) -> p k c", c=128)[:, :, 0:64]
                nc.vector.tensor_scalar_mul(
                    dst, tr[:, 0:256].rearrange("p (k c) -> p k c", c=64),
                    cf[:, VSC:VSC + 1])
                bal.charge("dve", 256)
            return f
        bg_tasks.append(mk_vtrans(0, v0))
        bg_tasks.append(mk_vtrans(1, v1))

    oT_tiles = {}

    def emit_div(J):
        """INLINE at chunk-J end: den reciprocal + broadcast + oT divide.
        Reads the pv psum tiles, so must precede the next pv acquisition."""
        oT = otp.tile([128, CK], BF16, tag="oT")
        oT_tiles[J] = oT
        pv0t, pv1t = pv_tiles.pop(J)
        rdB = []
        for hh, pvt in ((0, pv0t), (1, pv1t)):
            den = rdp.tile([1, CK], F32, tag=f"den{hh}")
            nc.vector.tensor_copy(den[:], pvt[64:65, :])
            rd = rdp.tile([1, CK], F32, tag=f"rd{hh}")
            nc.vector.reciprocal_approx_fast(rd[:], den[:])
            bal.charge("dve", 2 * CK)
            rb = rdp.tile([64, CK], F32, tag=f"rdB{hh}")
            nc.gpsimd.partition_broadcast(rb[:], rd[:], channels=64)
            rdB.append(rb)
        for hh, pvt in ((0, pv0t), (1, pv1t)):
            hsl = slice(hh * 64, (hh + 1) * 64)
            nc.vector.tensor_mul(oT[hsl, :], pvt[0:64, :], rdB[hh][:])
            bal.charge("dve", CK)

    def emit_outproj(J):
        """Queue chunk J's Wo matmuls + output DMA (oT(J) long ready by the
        time these drain, so they never block the PE FIFO)."""
        oT = oT_tiles.pop(J)

        def mk_dblk(dt_):
            def f():
                ps = ps_bg.tile([128, CK], F32, tag="bg")
                nc.tensor.matmul(ps[:], cb[:, W_O + dt_ * 128:W_O + (dt_ + 1) * 128],
                                 oT[:], start=True, stop=True)
                po = pop.tile([128, CK], BF16, tag="po")
                copy_psum(po[:], ps[:], CK)
                nc.sync.dma_start(
                    poT[dt_ * 128:(dt_ + 1) * 128, J * CK:(J + 1) * CK],
                    po[:])
            return f
        for dt_ in range(4):
            bg_tasks.append(mk_dblk(dt_))

    # ---------------- main pipeline ----------------
    pv_tiles = {}
    emit_proj(0)
    drain(3)  # chunk 0 q/k/v up front; its V-transposes overlap kb 0-1

    for J in range(NCH):
        if J + 1 < NCH:
            emit_proj(J + 1)
        if J >= 1:
            emit_outproj(J - 1)  # behind proj(J+1) tasks: oT(J-1) is ready
        pv0t = ps_pv.tile([128, CK], F32, tag="pv0")
        pv1t = ps_pv.tile([128, CK], F32, tag="pv1")
        pv_tiles[J] = (pv0t, pv1t)
        nkb = KB_PER_CK * (J + 1)
        qt = q_tiles.pop(J)
        pace["rate"] = (len(bg_tasks) + 1) / (2.0 * nkb)

        def emit_pv(kb):
            p = kb - KB_PER_CK * J
            col0 = KBLK * p if p >= 0 else 0
            for hh, vsb, pv in ((0, v0, pv0t), (1, v1, pv1t)):
                nc.tensor.matmul(
                    pv[:, col0:], vsb[:, kb * 128:(kb + 1) * 128],
                    kb_ets[kb][hh][:, col0:],
                    start=(kb == 0), stop=(kb == nkb - 1))
            del kb_ets[kb]

        kb_ets = {}
        for kb in range(nkb):
            p = kb - KB_PER_CK * J
            col0 = KBLK * p if p >= 0 else 0
            w = CK - col0
            sts, ets = [], []
            for hh in range(2):
                st = ps_st.tile([128, CK], F32, tag=f"st{hh}{kb % 2}")
                hsl = slice(hh * 64, (hh + 1) * 64)
                nc.tensor.matmul(
                    st[:, col0:], kT[hsl, kb * KBLK:(kb + 1) * KBLK],
                    qt[hsl, col0:], start=True, stop=True)
                sts.append(st)
            drain_paced()
            tail = (J == NCH - 1 and kb >= nkb - 4)
            for hh in range(2):
                et = etp.tile([128, CK], BF16, tag="et")
                c0 = col0
                if p >= 0:
                    # diagonal tile: fold the causal triangle into the exp
                    # itself - schraudolph with a [128,128] 2D bias holding
                    # bias + 0/-1e9 (replaces the separate tri multiply)
                    nc.vector.scalar_tensor_tensor(
                        et[:, c0:c0 + KBLK].bitcast(I16), sts[hh][:, c0:c0 + KBLK],
                        float(A128), cf[:, TRI16:TRI16 + KBLK], MULT, ADD)
                    bal.charge("dve", KBLK)
                    c0 += KBLK
                if c0 >= CK:
                    ets.append(et)
                    continue
                if tail:
                    # kernel tail: nothing left to overlap, so halve the
                    # exp latency by splitting across both engines
                    mid = (c0 + CK) // 2
                    nc.scalar.activation(
                        et[:, c0:mid], sts[hh][:, c0:mid], EXP,
                        bias=cf[:, BACT + kb:BACT + kb + 1], scale=0.125)
                    nc.vector.tensor_scalar(
                        et[:, mid:].bitcast(I16), sts[hh][:, mid:],
                        float(A128), cf[:, KB23 + kb:KB23 + kb + 1],
                        MULT, ADD)
                    bal.charge("act", mid - c0)
                    bal.charge("dve", CK - mid)
                elif bal.pick(CK - c0) == "act":
                    nc.scalar.activation(
                        et[:, c0:], sts[hh][:, c0:], EXP,
                        bias=cf[:, BACT + kb:BACT + kb + 1], scale=0.125)
                else:
                    nc.vector.tensor_scalar(
                        et[:, c0:].bitcast(I16), sts[hh][:, c0:],
                        float(A128), cf[:, KB23 + kb:KB23 + kb + 1],
                        MULT, ADD)
                ets.append(et)
            kb_ets[kb] = ets
            # software pipeline: PV(kb-1) sits BEHIND QK(kb) in the PE FIFO
            # so the PE streams QK(kb) while exp(kb-1) finishes
            if kb >= 1:
                emit_pv(kb - 1)
            drain_paced()
        emit_pv(nkb - 1)
        emit_div(J)
        drain(1)

    emit_outproj(NCH - 1)
    drain(99)


_CACHED = None


def _build():
    global _CACHED
    if _CACHED is not None:
        return _CACHED
    nc = bacc.Bacc("TRN2", target_bir_lowering=False, debug=False,
                   enable_asserts=False, num_devices=NCORES)
    names = [
        ("xT", [D, S], BF16), ("wpack", [128, 2240], BF16),
        ("fpack", [128, 196], F32),
    ]
    aps = [nc.dram_tensor(n, sh, dt_, kind="ExternalInput").ap()
           for n, sh, dt_ in names]
    poT = nc.dram_tensor("poT", [D, S], BF16, kind="ExternalOutput").ap()
    with tile.TileContext(nc) as tc, ExitStack() as ctx:
        _emit(nc, tc, ctx, aps + [poT])
    nc.compile()
    _CACHED = nc
    return nc


def _host_inputs(x, attention_mask, Wq, bq, Wk, bk, Wv, bv, Wo, bo):
    f = np.float32
    x = np.asarray(x, f)
    mask = np.asarray(attention_mask)
    Wq, Wk, Wv, Wo = (np.asarray(w, f) for w in (Wq, Wk, Wv, Wo))
    bq, bk, bv = (np.asarray(b_, f) for b_ in (bq, bk, bv))
    tri = np.triu(np.ones((128, 128), NPBF16))      # [k,q]: 1 where q >= k
    id2 = np.tile(np.eye(64, dtype=NPBF16), (2, 1))
    delta = ((np.arange(128) * PHI) % 1.0).astype(f)          # per key%128
    vscale = (2.0 ** -delta)[:, None].astype(f)
    in_maps = []
    for c in range(NCORES):
        b = c // 4
        h0 = 2 * (c % 4)
        hsl = slice(64 * h0, 64 * h0 + 128)

        def pack_w(W):
            wt = W[hsl, :].T                        # [512, 128] = Wh^T
            return np.ascontiguousarray(
                wt.reshape(4, 128, 128).transpose(1, 0, 2)
                .reshape(128, 512).astype(NPBF16))

        wo_t = Wo[:, hsl].T.astype(NPBF16)           # [128, 512]
        mk = np.where(mask[b] != 0, f(0.0), f(NEG)).astype(f)  # [S]
        mk = mk.reshape(32, 128).T                   # [128 part, 32 kb]
        kb23 = (128.0 * (127.0 + CSH) + 128.0 * delta)[:, None] + \
            np.where(mk < 0, f(-1e9), f(0.0))
        biasact = (delta * np.log(2.0))[:, None] + mk
        # diagonal-tile fused bias: schraudolph bias + causal 0/-1e9
        tri16 = np.where(np.triu(np.ones((128, 128), bool)),
                         kb23[:, 0:1], f(-1e9)).astype(f)
        wpack = np.concatenate(
            [pack_w(Wq), pack_w(Wk), pack_w(Wv), wo_t, tri, id2], axis=1)
        fpack = np.concatenate(
            [np.stack([bq[hsl], bk[hsl], bv[hsl]], axis=1).astype(f),
             kb23.astype(f), biasact.astype(f), vscale, tri16], axis=1)

        in_maps.append({
            "xT": np.ascontiguousarray(x[b].T.astype(NPBF16)),
            "wpack": np.ascontiguousarray(wpack),
            "fpack": np.ascontiguousarray(fpack),
        })
    return in_maps


def _assemble(results, bo):
    out = np.zeros((B, S, D), np.float32)
    for c in range(NCORES):
        out[c // 4] += results[c]["poT"].astype(np.float32).T
    out += np.asarray(bo, np.float32)
    return out


def kernel(**inputs) -> np.ndarray:
    nc = _build()
    in_maps = _host_inputs(**inputs)
    last_err = None
    for attempt in range(3):
        try:
            res = bass_utils.run_bass_kernel_spmd(
                nc, in_maps, core_ids=list(range(NCORES)))
            out = _assemble(res.results, inputs["bo"])
        except Exception as e:  # transient NRT/axon device errors
            last_err = e
            continue
        if np.isfinite(out).all():
            return out
        last_err = RuntimeError("non-finite output")
    raise last_err


def run_traced(inputs, **kwargs):
    """test.py helper: run with NTFF tracing, return (out, BassKernelResults)."""
    nc = _build()
    in_maps = _host_inputs(**inputs)
    res = bass_utils.run_bass_kernel_spmd(
        nc, in_maps, core_ids=list(range(NCORES)), trace=True, **kwargs)
    return _assemble(res.results, inputs["bo"]), res



# revision 25
# speedup vs baseline: 1.3612x; 1.0509x over previous
"""Causal self-attention (B=2, S=4096, D=512, H=8) on 8 Trainium2 cores.

Sharding: core c handles batch b = c//4 and heads {2*(c%4), 2*(c%4)+1}.

Design (v2): k-major flash-style attention with the exp() wall split across
TWO engines:
  - ScalarE computes exp natively (ACTIVATE, ~(N+352)/1.2 ns).
  - VectorE computes a one-instruction Schraudolph exp: writing
    int16(round(A*score + bias)) whose bit pattern IS the bf16 of
    2^(log2e*score/8 + delta): the exponent-bit trick computed directly in
    the >>16 scale.  Per-key exponent dither delta_r decorrelates the
    interpolation error; V rows (and the den ones-column) are pre-scaled by
    2^-delta_r on the host so the dither cancels exactly in PV.
A greedy ns-balancer assigns each score tile's exp (and the psum->sbuf
copies) to whichever of ACT/DVE is less loaded, so both engines run ~full
tilt alongside the TensorE stream.

Attention runs in 512-wide query chunks; projections for chunk J+1, the
output projection for chunk J-1, V transposes, and DMA are emitted as
background tasks interleaved between attention steps so PE never idles
(keeps the HAM clock at 2.4 GHz).  Denominators ride the PV matmul as a
65th 'ones' row; oT is divided on-device (reciprocal_approx_fast + gpsimd
partition broadcast) so the two heads fold into ONE output-projection pass
and the core writes a single [512, S] bf16 partial that the host sums.

PSUM budget (8 banks): pv0 pv1 | st x4 (score tiles, f32) | bg x2 (shared
by projections / V-transpose / out-projection).
"""

import sys

sys.path.insert(0, "/opt/trn_rl_repo")

from contextlib import ExitStack

import ml_dtypes
import numpy as np

import concourse.bass as bass
import concourse.tile as tile
from concourse import bacc, bass_utils, mybir

B, S, D = 2, 4096, 512
H, HD = 8, 64
NCORES = 8
F32 = mybir.dt.float32
BF16 = mybir.dt.bfloat16
I16 = mybir.dt.int16
FP8 = mybir.dt.float8e4
DR = mybir.MatmulPerfMode.DoubleRow
NPFP8 = ml_dtypes.float8_e4m3
EXP = mybir.ActivationFunctionType.Exp
IDENT = mybir.ActivationFunctionType.Identity
COPYF = mybir.ActivationFunctionType.Copy
MULT = mybir.AluOpType.mult
ADD = mybir.AluOpType.add
NPBF16 = ml_dtypes.bfloat16

CK = 512                      # query-chunk width
NCH = S // CK                 # 8
KBLK = 128                    # key block (partition dim)
KB_PER_CK = CK // KBLK        # 4
NEG = -1.0e30
LOG2E = 1.4426950408889634
A128 = 128 * LOG2E * 0.125    # DVE trick multiplier (raw-score units)
CSH = -0.045                  # Schraudolph shift
PHI = 0.6180339887498949


class Balancer:
    """Greedy ns-accounting across ACT and DVE for balanceable ops."""

    def __init__(self, nc):
        self.nc = nc
        self.ns = {"act": 2700.0, "dve": 0.0}  # ACT pays the exp table load

    def _cost(self, eng, w):
        return (w + 352) / 1.2 if eng == "act" else (w + 150) / 0.96

    def charge(self, eng, w):
        self.ns[eng] += self._cost(eng, w)

    def pick(self, w):
        eng = "act" if self.ns["act"] + self._cost("act", w) <= \
            self.ns["dve"] + self._cost("dve", w) else "dve"
        self.charge(eng, w)
        return eng


def _emit(nc, tc, ctx, io):
    xT, wpack, fpack, poT = io

    bal = Balancer(nc)

    const = ctx.enter_context(tc.tile_pool(name="const", bufs=1))
    sb = ctx.enter_context(tc.tile_pool(name="sb", bufs=1))

    # ---- constants / weights (two packed DMAs to keep the lead-in short) ----
    cb = const.tile([128, 2240], BF16, tag="cbf16")
    cf = const.tile([128, 196], F32, tag="cf32")
    nc.sync.dma_start(cb[:, 0:512], wpack[:, 0:512])
    nc.sync.dma_start(cb[:, 512:2240], wpack[:, 512:2240])
    nc.sync.dma_start(cf[:], fpack[:])
    W_Q, W_K, W_V, W_O, TRI, ID2 = 0, 512, 1024, 1536, 2048, 2176
    BQKV, KB23, BACT, VSC, TRI16 = 0, 3, 35, 67, 68

    # ---- persistent SBUF ----
    kT = sb.tile([128, S], BF16, tag="kT")       # [2*64 hd, keys]
    # k-major V blocks padded to 128 cols (hd | ones@64 | junk pad) so the
    # PV LDWEIGHTS takes the full-width fast path; pv rows 65+ are junk
    v0 = sb.tile([128, 32 * 128], BF16, tag="v0")
    v1 = sb.tile([128, 32 * 128], BF16, tag="v1")

    xin = ctx.enter_context(tc.tile_pool(name="xin", bufs=2))
    qp = ctx.enter_context(tc.tile_pool(name="qp", bufs=2))
    vtp = ctx.enter_context(tc.tile_pool(name="vtp", bufs=2))
    etp = ctx.enter_context(tc.tile_pool(name="etp", bufs=10))
    otp = ctx.enter_context(tc.tile_pool(name="otp", bufs=2))
    pop = ctx.enter_context(tc.tile_pool(name="pop", bufs=4))
    rdp = ctx.enter_context(tc.tile_pool(name="rdp", bufs=2))

    ps_pv = ctx.enter_context(tc.tile_pool(name="ps_pv", bufs=1, space="PSUM"))
    ps_st = ctx.enter_context(tc.tile_pool(name="ps_st", bufs=1, space="PSUM"))
    ps_bg = ctx.enter_context(tc.tile_pool(name="ps_bg", bufs=2, space="PSUM"))

    # ones columns of v0/v1 (scaled 2^-delta); written once, blocks fill later
    for vdst in (v0, v1):
        ones_col = vdst[:].rearrange("p (k c) -> p k c", c=128)[:, :, 64:65]
        nc.vector.tensor_copy(ones_col, cf[:, VSC:VSC + 1].to_broadcast((128, 32, 1)))

    # ---------------- background task machinery ----------------
    bg_tasks = []
    pace = {"credit": 0.0, "rate": 1.0}

    def drain(n):
        for _ in range(min(n, len(bg_tasks))):
            bg_tasks.pop(0)()

    def drain_paced():
        """Spread queued tasks over the chunk's drain slots so the PE always
        has background work, even late in a chunk."""
        pace["credit"] += pace["rate"]
        while pace["credit"] >= 1.0 and bg_tasks:
            pace["credit"] -= 1.0
            bg_tasks.pop(0)()

    def copy_psum(dst_ap, src_ap, w, bias_col=None, scale=1.0):
        """psum->sbuf evacuation on the less-loaded of ACT/DVE."""
        eng = bal.pick(w)
        if eng == "act":
            if bias_col is not None:
                nc.scalar.activation(dst_ap, src_ap, IDENT, bias=bias_col,
                                     scale=scale)
            else:
                nc.scalar.copy(dst_ap, src_ap)
        else:
            if bias_col is not None:
                nc.vector.tensor_scalar(dst_ap, src_ap, scale, bias_col,
                                        MULT, ADD)
            else:
                nc.vector.tensor_copy(dst_ap, src_ap)

    q_tiles = {}

    def emit_proj(J):
        """q/k/v projections for chunk J + V transpose to k-major."""
        xt = xin.tile([128, 4 * CK], BF16, tag="x")
        for ks in range(4):
            nc.sync.dma_start(
                xt[:, ks * CK:(ks + 1) * CK],
                xT[ks * 128:(ks + 1) * 128, J * CK:(J + 1) * CK])
        qt = qp.tile([128, CK], BF16, tag="q")
        q_tiles[J] = qt
        vt = vtp.tile([128, CK], BF16, tag="v")
        csl = slice(J * CK, (J + 1) * CK)

        def mk_proj(woff, bcol, dst_ap):
            def f():
                ps = ps_bg.tile([128, CK], F32, tag="bg")
                for ks in range(4):
                    nc.tensor.matmul(
                        ps[:], cb[:, woff + ks * 128:woff + (ks + 1) * 128],
                        xt[:, ks * CK:(ks + 1) * CK],
                        start=(ks == 0), stop=(ks == 3))
                copy_psum(dst_ap, ps[:], CK,
                          bias_col=cf[:, BQKV + bcol:BQKV + bcol + 1])
            return f

        bg_tasks.append(mk_proj(W_Q, 0, qt[:]))
        bg_tasks.append(mk_proj(W_K, 1, kT[:, csl]))
        bg_tasks.append(mk_proj(W_V, 2, vt[:]))

        def mk_vtrans(hh, vdst):
            def f():
                # own psum buffer per head: a shared bank would let head0's
                # DVE copy (bank read) overlap head1's PE transposes (bank
                # write) -> fatal PSUM collision
                ps = ps_bg.tile([128, CK], F32, tag="bg")
                tr = ps[:].bitcast(BF16)  # [128, 1024] bf16 view
                for i in range(4):
                    nc.tensor.transpose(
                        tr[:, i * 64:(i + 1) * 64],
                        vt[hh * 64:(hh + 1) * 64, i * KBLK:(i + 1) * KBLK],
                        cb[hh * 64:(hh + 1) * 64, ID2:ID2 + 64])
                dst = vdst[:, (J * 4) * 128:(J * 4 + 4) * 128]
                dst = dst.rearrange("p (k c) -> p k c", c=128)[:, :, 0:64]
                nc.vector.tensor_scalar_mul(
                    dst, tr[:, 0:256].rearrange("p (k c) -> p k c", c=64),
                    cf[:, VSC:VSC + 1])
                bal.charge("dve", 256)
            return f
        bg_tasks.append(mk_vtrans(0, v0))
        bg_tasks.append(mk_vtrans(1, v1))

    oT_tiles = {}

    def emit_div(J):
        """INLINE at chunk-J end: den reciprocal + broadcast + oT divide.
        Reads the pv psum tiles, so must precede the next pv acquisition."""
        oT = otp.tile([128, CK], BF16, tag="oT")
        oT_tiles[J] = oT
        pv0t, pv1t = pv_tiles.pop(J)
        rdB = []
        for hh, pvt in ((0, pv0t), (1, pv1t)):
            den = rdp.tile([1, CK], F32, tag=f"den{hh}")
            nc.vector.tensor_copy(den[:], pvt[64:65, :])
            rd = rdp.tile([1, CK], F32, tag=f"rd{hh}")
            nc.vector.reciprocal_approx_fast(rd[:], den[:])
            bal.charge("dve", 2 * CK)
            rb = rdp.tile([64, CK], F32, tag=f"rdB{hh}")
            nc.gpsimd.partition_broadcast(rb[:], rd[:], channels=64)
            rdB.append(rb)
        for hh, pvt in ((0, pv0t), (1, pv1t)):
            hsl = slice(hh * 64, (hh + 1) * 64)
            nc.vector.tensor_mul(oT[hsl, :], pvt[0:64, :], rdB[hh][:])
            bal.charge("dve", CK)

    def emit_outproj(J):
        """Queue chunk J's Wo matmuls + output DMA (oT(J) long ready by the
        time these drain, so they never block the PE FIFO)."""
        oT = oT_tiles.pop(J)

        def mk_dblk(dt_):
            def f():
                ps = ps_bg.tile([128, CK], F32, tag="bg")
                nc.tensor.matmul(ps[:], cb[:, W_O + dt_ * 128:W_O + (dt_ + 1) * 128],
                                 oT[:], start=True, stop=True)
                po = pop.tile([128, CK], BF16, tag="po")
                copy_psum(po[:], ps[:], CK)
                nc.sync.dma_start(
                    poT[dt_ * 128:(dt_ + 1) * 128, J * CK:(J + 1) * CK],
                    po[:])
            return f
        for dt_ in range(4):
            bg_tasks.append(mk_dblk(dt_))

    # ---------------- main pipeline ----------------
    pv_tiles = {}
    emit_proj(0)
    drain(3)  # chunk 0 q/k/v up front; its V-transposes overlap kb 0-1

    for J in range(NCH):
        if J + 1 < NCH:
            emit_proj(J + 1)
        if J >= 1:
            emit_outproj(J - 1)  # behind proj(J+1) tasks: oT(J-1) is ready
        pv0t = ps_pv.tile([128, CK], F32, tag="pv0")
        pv1t = ps_pv.tile([128, CK], F32, tag="pv1")
        pv_tiles[J] = (pv0t, pv1t)
        nkb = KB_PER_CK * (J + 1)
        qt = q_tiles.pop(J)
        pace["rate"] = (len(bg_tasks) + 1) / (1.0 * nkb)

        def emit_pv(kb):
            p = kb - KB_PER_CK * J
            col0 = KBLK * p if p >= 0 else 0
            for hh, vsb, pv in ((0, v0, pv0t), (1, v1, pv1t)):
                nc.tensor.matmul(
                    pv[:, col0:], vsb[:, kb * 128:(kb + 1) * 128],
                    kb_ets[kb][hh][:, col0:],
                    start=(kb == 0), stop=(kb == nkb - 1))
            del kb_ets[kb]

        kb_ets = {}
        npair = nkb // 2
        for pr in range(npair):
          # ---- 64-row-mode phase: QK for both kbs of the pair ----
          # (the two heads' 64-partition QK matmuls run concurrently in the
          # two row tiles; batching 2 kbs per mode round-trip halves the PE
          # drains paid on 64<->128 tiling-mode changes)
          kb_sts = {}
          for kb in (2 * pr, 2 * pr + 1):
            p = kb - KB_PER_CK * J
            col0 = KBLK * p if p >= 0 else 0
            sts = []
            for hh in range(2):
                st = ps_st.tile([128, CK], F32, tag=f"st{hh}{kb % 2}")
                hsl = slice(hh * 64, (hh + 1) * 64)
                nc.tensor.matmul(
                    st[:, col0:], kT[hsl, kb * KBLK:(kb + 1) * KBLK],
                    qt[hsl, col0:], start=True, stop=True)
                sts.append(st)
            kb_sts[kb] = sts
          drain_paced()
          for kb in (2 * pr, 2 * pr + 1):
            p = kb - KB_PER_CK * J
            col0 = KBLK * p if p >= 0 else 0
            w = CK - col0
            sts = kb_sts[kb]
            ets = []
            tail = (J == NCH - 1 and kb >= nkb - 4)
            for hh in range(2):
                et = etp.tile([128, CK], BF16, tag="et")
                c0 = col0
                if p >= 0:
                    # diagonal tile: fold the causal triangle into the exp
                    # itself - schraudolph with a [128,128] 2D bias holding
                    # bias + 0/-1e9 (replaces the separate tri multiply)
                    nc.vector.scalar_tensor_tensor(
                        et[:, c0:c0 + KBLK].bitcast(I16), sts[hh][:, c0:c0 + KBLK],
                        float(A128), cf[:, TRI16:TRI16 + KBLK], MULT, ADD)
                    bal.charge("dve", KBLK)
                    c0 += KBLK
                if c0 >= CK:
                    ets.append(et)
                    continue
                if tail:
                    # kernel tail: nothing left to overlap, so halve the
                    # exp latency by splitting across both engines
                    mid = (c0 + CK) // 2
                    nc.scalar.activation(
                        et[:, c0:mid], sts[hh][:, c0:mid], EXP,
                        bias=cf[:, BACT + kb:BACT + kb + 1], scale=0.125)
                    nc.vector.tensor_scalar(
                        et[:, mid:].bitcast(I16), sts[hh][:, mid:],
                        float(A128), cf[:, KB23 + kb:KB23 + kb + 1],
                        MULT, ADD)
                    bal.charge("act", mid - c0)
                    bal.charge("dve", CK - mid)
                elif bal.pick(CK - c0) == "act":
                    nc.scalar.activation(
                        et[:, c0:], sts[hh][:, c0:], EXP,
                        bias=cf[:, BACT + kb:BACT + kb + 1], scale=0.125)
                else:
                    nc.vector.tensor_scalar(
                        et[:, c0:].bitcast(I16), sts[hh][:, c0:],
                        float(A128), cf[:, KB23 + kb:KB23 + kb + 1],
                        MULT, ADD)
                ets.append(et)
            kb_ets[kb] = ets
          # ---- 128-mode phase: PV of the previous pair (its exps are
          # done by now; software pipeline lag = one pair) ----
          if pr >= 1:
            emit_pv(2 * pr - 2)
            emit_pv(2 * pr - 1)
          drain_paced()
        emit_pv(nkb - 2)
        emit_pv(nkb - 1)
        emit_div(J)
        drain(1)

    emit_outproj(NCH - 1)
    drain(99)


_CACHED = None


def _build():
    global _CACHED
    if _CACHED is not None:
        return _CACHED
    nc = bacc.Bacc("TRN2", target_bir_lowering=False, debug=False,
                   enable_asserts=False, num_devices=NCORES)
    names = [
        ("xT", [D, S], BF16), ("wpack", [128, 2240], BF16),
        ("fpack", [128, 196], F32),
    ]
    aps = [nc.dram_tensor(n, sh, dt_, kind="ExternalInput").ap()
           for n, sh, dt_ in names]
    poT = nc.dram_tensor("poT", [D, S], BF16, kind="ExternalOutput").ap()
    with tile.TileContext(nc) as tc, ExitStack() as ctx:
        _emit(nc, tc, ctx, aps + [poT])
    nc.compile()
    _CACHED = nc
    return nc


def _host_inputs(x, attention_mask, Wq, bq, Wk, bk, Wv, bv, Wo, bo):
    f = np.float32
    x = np.asarray(x, f)
    mask = np.asarray(attention_mask)
    Wq, Wk, Wv, Wo = (np.asarray(w, f) for w in (Wq, Wk, Wv, Wo))
    bq, bk, bv = (np.asarray(b_, f) for b_ in (bq, bk, bv))
    tri = np.triu(np.ones((128, 128), NPBF16))      # [k,q]: 1 where q >= k
    id2 = np.tile(np.eye(64, dtype=NPBF16), (2, 1))
    delta = ((np.arange(128) * PHI) % 1.0).astype(f)          # per key%128
    vscale = (2.0 ** -delta)[:, None].astype(f)
    in_maps = []
    for c in range(NCORES):
        b = c // 4
        h0 = 2 * (c % 4)
        hsl = slice(64 * h0, 64 * h0 + 128)

        def pack_w(W):
            wt = W[hsl, :].T                        # [512, 128] = Wh^T
            return np.ascontiguousarray(
                wt.reshape(4, 128, 128).transpose(1, 0, 2)
                .reshape(128, 512).astype(NPBF16))

        wo_t = Wo[:, hsl].T.astype(NPBF16)           # [128, 512]
        mk = np.where(mask[b] != 0, f(0.0), f(NEG)).astype(f)  # [S]
        mk = mk.reshape(32, 128).T                   # [128 part, 32 kb]
        kb23 = (128.0 * (127.0 + CSH) + 128.0 * delta)[:, None] + \
            np.where(mk < 0, f(-1e9), f(0.0))
        biasact = (delta * np.log(2.0))[:, None] + mk
        # diagonal-tile fused bias: schraudolph bias + causal 0/-1e9
        tri16 = np.where(np.triu(np.ones((128, 128), bool)),
                         kb23[:, 0:1], f(-1e9)).astype(f)
        wpack = np.concatenate(
            [pack_w(Wq), pack_w(Wk), pack_w(Wv), wo_t, tri, id2], axis=1)
        fpack = np.concatenate(
            [np.stack([bq[hsl], bk[hsl], bv[hsl]], axis=1).astype(f),
             kb23.astype(f), biasact.astype(f), vscale, tri16], axis=1)

        in_maps.append({
            "xT": np.ascontiguousarray(x[b].T.astype(NPBF16)),
            "wpack": np.ascontiguousarray(wpack),
            "fpack": np.ascontiguousarray(fpack),
        })
    return in_maps


def _assemble(results, bo):
    out = np.zeros((B, S, D), np.float32)
    for c in range(NCORES):
        out[c // 4] += results[c]["poT"].astype(np.float32).T
    out += np.asarray(bo, np.float32)
    return out


def kernel(**inputs) -> np.ndarray:
    nc = _build()
    in_maps = _host_inputs(**inputs)
    last_err = None
    for attempt in range(3):
        try:
            res = bass_utils.run_bass_kernel_spmd(
                nc, in_maps, core_ids=list(range(NCORES)))
            out = _assemble(res.results, inputs["bo"])
        except Exception as e:  # transient NRT/axon device errors
            last_err = e
            continue
        if np.isfinite(out).all():
            return out
        last_err = RuntimeError("non-finite output")
    raise last_err


def run_traced(inputs, **kwargs):
    """test.py helper: run with NTFF tracing, return (out, BassKernelResults)."""
    nc = _build()
    in_maps = _host_inputs(**inputs)
    res = bass_utils.run_bass_kernel_spmd(
        nc, in_maps, core_ids=list(range(NCORES)), trace=True, **kwargs)
    return _assemble(res.results, inputs["bo"]), res

